# revision 23
# baseline (speedup 1.0000x reference)
"""Distributed L1-attention dictionary lookup (retrieval_knn) on 8 trn2 cores.

out = sigmoid(softmax(-sum|keys - q|) @ values)

Sharding: capacity (262144) split row-wise into 8 shards of 32768. Each core:
  t_c   = sum_f |k_cf - q_f|            (DVE subtract + ACT abs-accumulate)
  tmin  = min_c t_c                     (DVE reduce + PE transpose + DVE reduce)
  att_c = exp(tmin - t_c)               (ACT exp, bias = broadcast tmin)
  num   = sum_c att_c * v_c   [512]     (PE, 256 accumulating matmuls)
  s     = sum_c att_c
Host merges 8 (num, s, tmin) triples with a stable log-sum-exp combine and
applies the final divide + sigmoid.
"""

from contextlib import ExitStack

import numpy as np

import concourse.bacc as bacc
import concourse.bass as bass
import concourse.mybir as mybir
import concourse.tile as tile
from concourse.bass_utils import run_bass_kernel_spmd

F32 = mybir.dt.float32

NCORES = 8
CAP = 262144
F = 512
SHARD = CAP // NCORES  # 32768

# Streaming geometry: one chunk = CHUNK_ROWS key/value rows laid out so each
# SBUF partition holds CHUNK_ROWS/128 *consecutive* DRAM rows (contiguous
# per-partition DMA descriptors). Keys and values use the same permutation, so
# the softmax/matmul pairing stays consistent.
CHUNK_ROWS = 512
SUBT = CHUNK_ROWS // 128  # sub-tiles ([128,512] slabs) per chunk


MM_MODE = "f32"  # "f32" | "f32r"
NACT = 4  # sub-tiles/chunk whose abs+reduce runs on ScalarE; rest on VectorE
KBUFS = 3
VBUFS = 3
DBUFS = 3
SKIP_COMPUTE_A = False  # DMA keys but skip score compute (perf probes)
SKIP_STATS = False
FAKE_MIN_CHAIN = False  # exp with immediate bias (perf probe)
SKIP_PHASE_B = False


FUSED = True


def _body_fused(ctx, tc, q, k, v, ident, ones, out_vec, out_stats, shard_rows, chunk_rows):
    """Single fused stream: keys and values chunks flow together.

    The softmax reference point m is the min over chunk 0's scores (any
    reference is mathematically valid; host combine only needs consistency).
    exp(m - t) <= exp(m - min t) stays far below fp32 max (~e^20 worst case),
    so per-chunk exp + matmul can run as soon as each chunk's scores land —
    no global barrier, both DMA streams stay saturated end to end.
    """
    nc = tc.nc
    subt = chunk_rows // 128
    nchunk = shard_rows // chunk_rows
    ncols = shard_rows // 128
    nact = min(NACT, subt)

    singles = ctx.enter_context(tc.tile_pool(name="singles", bufs=1))
    kpool = ctx.enter_context(tc.tile_pool(name="kpool", bufs=KBUFS))
    vpool = ctx.enter_context(tc.tile_pool(name="vpool", bufs=VBUFS))
    dpool = ctx.enter_context(tc.tile_pool(name="dpool", bufs=DBUFS))
    apool = ctx.enter_context(tc.tile_pool(name="apool", bufs=4))
    pp = ctx.enter_context(tc.tile_pool(name="pp", bufs=1, space="PSUM"))

    qb = singles.tile([128, F], F32, tag="qb")
    nc.sync.dma_start(
        out=qb,
        in_=bass.AP(tensor=q.tensor, offset=q.offset, ap=[[0, 128]] + list(q.ap)),
    )
    id_sb = singles.tile([128, 128], F32, tag="id")
    nc.sync.dma_start(out=id_sb, in_=ident)
    ones_sb = singles.tile([1, 128], F32, tag="ones")
    nc.sync.dma_start(out=ones_sb, in_=ones)

    scores = singles.tile([128, ncols], F32, tag="scores")
    att = singles.tile([128, ncols], F32, tag="att")
    gmin = singles.tile([1, 1], F32, tag="gmin")
    gmin_col = singles.tile([128, 1], F32, tag="gmincol")

    kt = k.rearrange("(n p j) f -> n p j f", p=128, j=subt)
    vt = v.rearrange("(n p j) f -> n p j f", p=128, j=subt)

    acc = pp.tile([1, F], F32, tag="acc")
    for n in range(nchunk):
        kc = kpool.tile([128, subt, F], F32, tag="kc")
        nc.sync.dma_start(out=kc, in_=kt[n])
        vc = vpool.tile([128, subt, F], F32, tag="vc")
        nc.scalar.dma_start(out=vc, in_=vt[n])

        dchunk = dpool.tile([128, subt, F], F32, tag="dch")
        for j in range(subt):
            nc.vector.tensor_tensor(
                out=dchunk[:, j], in0=kc[:, j], in1=qb, op=mybir.AluOpType.subtract
            )
        col0 = n * subt
        for j in range(nact):
            a = apool.tile([128, F], F32, tag="a")
            nc.scalar.activation(
                out=a,
                in_=dchunk[:, j],
                func=mybir.ActivationFunctionType.Abs,
                accum_out=scores[:, col0 + j : col0 + j + 1],
            )
        if nact < subt:
            nc.vector.tensor_reduce(
                out=scores[:, col0 + nact : col0 + subt],
                in_=dchunk[:, nact:subt],
                axis=mybir.AxisListType.X,
                op=mybir.AluOpType.add,
                apply_absolute_value=True,
            )

        if n == 0:
            # softmax reference point from chunk 0 only
            tmin = singles.tile([128, 1], F32, tag="tmin")
            nc.vector.tensor_reduce(
                out=tmin,
                in_=scores[:, 0:subt],
                axis=mybir.AxisListType.X,
                op=mybir.AluOpType.min,
            )
            trow_ps = pp.tile([1, 128], F32, tag="trow")
            nc.tensor.transpose(trow_ps, tmin, id_sb)
            nc.vector.tensor_reduce(
                out=gmin,
                in_=trow_ps,
                axis=mybir.AxisListType.X,
                op=mybir.AluOpType.min,
            )
            bc_ps = pp.tile([128, 1], F32, tag="bc")
            nc.tensor.matmul(bc_ps, lhsT=ones_sb, rhs=gmin, start=True, stop=True)
            nc.scalar.copy(out=gmin_col, in_=bc_ps)

        nc.scalar.activation(
            out=att[:, col0 : col0 + subt],
            in_=scores[:, col0 : col0 + subt],
            func=mybir.ActivationFunctionType.Exp,
            bias=gmin_col,
            scale=-1.0,
        )
        for j in range(subt):
            c = col0 + j
            lhs_ap = att[:, c : c + 1]
            rhs_ap = vc[:, j]
            if MM_MODE == "f32r":
                lhs_ap = lhs_ap.bitcast(mybir.dt.float32r)
                rhs_ap = rhs_ap.bitcast(mybir.dt.float32r)
            nc.tensor.matmul(
                acc,
                lhsT=lhs_ap,
                rhs=rhs_ap,
                start=(c == 0),
                stop=(c == ncols - 1),
            )

    # ---- tail: s = sum(att), pack outputs ----
    scol = singles.tile([128, 1], F32, tag="scol")
    nc.vector.tensor_reduce(
        out=scol, in_=att, axis=mybir.AxisListType.X, op=mybir.AluOpType.add
    )
    srow_ps = pp.tile([1, 128], F32, tag="srow")
    nc.tensor.transpose(srow_ps, scol, id_sb)
    ssum = singles.tile([1, 1], F32, tag="ssum")
    nc.vector.tensor_reduce(
        out=ssum, in_=srow_ps, axis=mybir.AxisListType.X, op=mybir.AluOpType.add
    )
    out_sb = singles.tile([1, F], F32, tag="outsb")
    nc.vector.tensor_copy(out=out_sb, in_=acc)
    st_sb = singles.tile([1, 2], F32, tag="stsb")
    nc.vector.tensor_copy(out=st_sb[:, 0:1], in_=ssum)
    nc.vector.tensor_copy(out=st_sb[:, 1:2], in_=gmin)
    nc.sync.dma_start(out=out_vec, in_=out_sb)
    nc.sync.dma_start(out=out_stats, in_=st_sb)


def _body(ctx, tc, q, k, v, ident, ones, out_vec, out_stats, shard_rows, chunk_rows):
    if FUSED:
        return _body_fused(
            ctx, tc, q, k, v, ident, ones, out_vec, out_stats, shard_rows, chunk_rows
        )
    nc = tc.nc
    subt = chunk_rows // 128
    nchunk = shard_rows // chunk_rows
    ncols = shard_rows // 128  # score columns

    singles = ctx.enter_context(tc.tile_pool(name="singles", bufs=1))
    kpool = ctx.enter_context(tc.tile_pool(name="kpool", bufs=3))
    vpool = ctx.enter_context(tc.tile_pool(name="vpool", bufs=3))
    dpool = ctx.enter_context(tc.tile_pool(name="dpool", bufs=3))
    apool = ctx.enter_context(tc.tile_pool(name="apool", bufs=4))
    pp = ctx.enter_context(tc.tile_pool(name="pp", bufs=1, space="PSUM"))

    # query broadcast to all 128 partitions (stride-0 partition DMA)
    qb = singles.tile([128, F], F32, tag="qb")
    nc.sync.dma_start(
        out=qb,
        in_=bass.AP(tensor=q.tensor, offset=q.offset, ap=[[0, 128]] + list(q.ap)),
    )
    id_sb = singles.tile([128, 128], F32, tag="id")
    nc.sync.dma_start(out=id_sb, in_=ident)
    ones_sb = singles.tile([1, 128], F32, tag="ones")
    nc.sync.dma_start(out=ones_sb, in_=ones)

    scores = singles.tile([128, ncols], F32, tag="scores")
    if SKIP_COMPUTE_A:
        nc.vector.memset(scores, 1.0)

    kt = k.rearrange("(n p j) f -> n p j f", p=128, j=subt)
    vt = v.rearrange("(n p j) f -> n p j f", p=128, j=subt)

    # ---- Phase A: stream keys, compute t_c = sum_f |k - q| ----
    for n in range(nchunk):
        kc = kpool.tile([128, subt, F], F32, tag="kc")
        nc.sync.dma_start(out=kc, in_=kt[n])
        if SKIP_COMPUTE_A:
            continue
        nact = min(NACT, subt)
        dchunk = dpool.tile([128, subt, F], F32, tag="dch")
        for j in range(subt):
            nc.vector.tensor_tensor(
                out=dchunk[:, j], in0=kc[:, j], in1=qb, op=mybir.AluOpType.subtract
            )
        for j in range(nact):
            c = n * subt + j
            a = apool.tile([128, F], F32, tag="a")
            nc.scalar.activation(
                out=a,
                in_=dchunk[:, j],
                func=mybir.ActivationFunctionType.Abs,
                accum_out=scores[:, c : c + 1],
            )
        if nact < subt:
            nc.vector.tensor_reduce(
                out=scores[:, n * subt + nact : n * subt + subt],
                in_=dchunk[:, nact:subt],
                axis=mybir.AxisListType.X,
                op=mybir.AluOpType.add,
                apply_absolute_value=True,
            )

    # ---- Stats: global min of t, broadcast, exponentiate ----
    att = singles.tile([128, ncols], F32, tag="att")
    gmin = singles.tile([1, 1], F32, tag="gmin")
    ssum = singles.tile([1, 1], F32, tag="ssum")
    if SKIP_STATS:
        nc.vector.memset(att, 0.5)
        nc.vector.memset(gmin, 0.0)
        nc.vector.memset(ssum, 1.0)
    else:
        tmin = singles.tile([128, 1], F32, tag="tmin")
        nc.vector.tensor_reduce(
            out=tmin, in_=scores, axis=mybir.AxisListType.X, op=mybir.AluOpType.min
        )
        trow_ps = pp.tile([1, 128], F32, tag="trow")
        nc.tensor.transpose(trow_ps, tmin, id_sb)
        nc.vector.tensor_reduce(
            out=gmin, in_=trow_ps, axis=mybir.AxisListType.X, op=mybir.AluOpType.min
        )
        bc_ps = pp.tile([128, 1], F32, tag="bc")
        nc.tensor.matmul(bc_ps, lhsT=ones_sb, rhs=gmin, start=True, stop=True)
        gmin_col = singles.tile([128, 1], F32, tag="gmincol")
        nc.scalar.copy(out=gmin_col, in_=bc_ps)

        nc.scalar.activation(
            out=att,
            in_=scores,
            func=mybir.ActivationFunctionType.Exp,
            bias=0.0 if FAKE_MIN_CHAIN else gmin_col,
            scale=-1.0,
        )
        scol = singles.tile([128, 1], F32, tag="scol")
        nc.vector.tensor_reduce(
            out=scol, in_=att, axis=mybir.AxisListType.X, op=mybir.AluOpType.add
        )
        srow_ps = pp.tile([1, 128], F32, tag="srow")
        nc.tensor.transpose(srow_ps, scol, id_sb)
        nc.vector.tensor_reduce(
            out=ssum, in_=srow_ps, axis=mybir.AxisListType.X, op=mybir.AluOpType.add
        )

    # ---- Phase B: stream values, accumulate att @ V into one PSUM bank ----
    acc = None if SKIP_PHASE_B else pp.tile([1, F], F32, tag="acc")
    for n in range(nchunk):
        vc = vpool.tile([128, subt, F], F32, tag="vc")
        nc.scalar.dma_start(out=vc, in_=vt[n])
        if SKIP_PHASE_B:
            continue
        for j in range(subt):
            c = n * subt + j
            lhs_ap = att[:, c : c + 1]
            rhs_ap = vc[:, j]
            if MM_MODE == "f32r":
                lhs_ap = lhs_ap.bitcast(mybir.dt.float32r)
                rhs_ap = rhs_ap.bitcast(mybir.dt.float32r)
            nc.tensor.matmul(
                acc,
                lhsT=lhs_ap,
                rhs=rhs_ap,
                start=(c == 0),
                stop=(c == ncols - 1),
            )

    out_sb = singles.tile([1, F], F32, tag="outsb")
    if SKIP_PHASE_B:
        nc.vector.memset(out_sb, 0.0)
    else:
        nc.vector.tensor_copy(out=out_sb, in_=acc)
    st_sb = singles.tile([1, 2], F32, tag="stsb")
    nc.vector.tensor_copy(out=st_sb[:, 0:1], in_=ssum)
    nc.vector.tensor_copy(out=st_sb[:, 1:2], in_=gmin)
    nc.sync.dma_start(out=out_vec, in_=out_sb)
    nc.sync.dma_start(out=out_stats, in_=st_sb)


def build_nc(shard_rows=SHARD, chunk_rows=CHUNK_ROWS, num_devices=NCORES, reps=1):
    nc = bacc.Bacc(
        "TRN2", target_bir_lowering=False, debug=False, num_devices=num_devices
    )
    q_h = nc.dram_tensor("query", [F], F32, kind="ExternalInput")
    k_h = nc.dram_tensor("keys", [shard_rows, F], F32, kind="ExternalInput")
    v_h = nc.dram_tensor("values", [shard_rows, F], F32, kind="ExternalInput")
    id_h = nc.dram_tensor("ident", [128, 128], F32, kind="ExternalInput")
    ones_h = nc.dram_tensor("ones_row", [1, 128], F32, kind="ExternalInput")
    onum_h = nc.dram_tensor("out_vec", [1, F], F32, kind="ExternalOutput")
    ostat_h = nc.dram_tensor("out_stats", [1, 2], F32, kind="ExternalOutput")

    with tile.TileContext(nc) as tc, ExitStack() as ctx:
        for _ in range(reps):
            with ExitStack() as rep_ctx:
                _body(
                    rep_ctx,
                    tc,
                    q_h.ap(),
                    k_h.ap(),
                    v_h.ap(),
                    id_h.ap(),
                    ones_h.ap(),
                    onum_h.ap(),
                    ostat_h.ap(),
                    shard_rows,
                    chunk_rows,
                )
    nc.compile()
    return nc


def make_in_maps(query, keys, values, shard_rows=SHARD, ncores=NCORES):
    query = np.ascontiguousarray(np.asarray(query), dtype=np.float32)
    keys = np.asarray(keys)
    values = np.asarray(values)
    ident = np.eye(128, dtype=np.float32)
    ones = np.ones((1, 128), dtype=np.float32)
    in_maps = []
    for i in range(ncores):
        sl = slice(i * shard_rows, (i + 1) * shard_rows)
        in_maps.append(
            {
                "query": query,
                "keys": np.ascontiguousarray(keys[sl], dtype=np.float32),
                "values": np.ascontiguousarray(values[sl], dtype=np.float32),
                "ident": ident,
                "ones_row": ones,
            }
        )
    return in_maps


def combine(results):
    """Merge per-core (num, s, tmin) partials: stable cross-shard softmax."""
    num = np.stack([np.asarray(r["out_vec"])[0] for r in results]).astype(np.float64)
    st = np.stack([np.asarray(r["out_stats"])[0] for r in results]).astype(np.float64)
    s, tmin = st[:, 0], st[:, 1]
    t0 = tmin.min()
    w = np.exp(t0 - tmin)  # <= 1
    vec = (num * w[:, None]).sum(axis=0) / (s * w).sum()
    return (1.0 / (1.0 + np.exp(-vec))).astype(np.float32)


_NC_CACHE = None


def kernel(query, keys, values):
    global _NC_CACHE
    if _NC_CACHE is None:
        _NC_CACHE = build_nc()
    in_maps = make_in_maps(query, keys, values)
    res = run_bass_kernel_spmd(_NC_CACHE, in_maps, core_ids=list(range(NCORES)))
    return combine(res.results)


if __name__ == "__main__":
    rng = np.random.default_rng(0)
    q = rng.standard_normal(F).astype(np.float32)
    k = rng.standard_normal((CAP, F)).astype(np.float32)
    v = rng.standard_normal((CAP, F)).astype(np.float32)
    out = kernel(q, k, v)
    print(out[:8])


# revision 24
# speedup vs baseline: 1.1178x; 1.1178x over previous
"""Distributed L1-attention dictionary lookup (retrieval_knn) on 8 trn2 cores.

out = sigmoid(softmax(-sum|keys - q|) @ values)

Sharding: capacity (262144) split row-wise into 8 shards of 32768. Each core:
  t_c   = sum_f |k_cf - q_f|            (DVE subtract + ACT abs-accumulate)
  tmin  = min_c t_c                     (DVE reduce + PE transpose + DVE reduce)
  att_c = exp(tmin - t_c)               (ACT exp, bias = broadcast tmin)
  num   = sum_c att_c * v_c   [512]     (PE, 256 accumulating matmuls)
  s     = sum_c att_c
Host merges 8 (num, s, tmin) triples with a stable log-sum-exp combine and
applies the final divide + sigmoid.
"""

from contextlib import ExitStack

import numpy as np

import concourse.bacc as bacc
import concourse.bass as bass
import concourse.mybir as mybir
import concourse.tile as tile
from concourse.bass_utils import run_bass_kernel_spmd

F32 = mybir.dt.float32

NCORES = 8
CAP = 262144
F = 512
SHARD = CAP // NCORES  # 32768

# Streaming geometry: one chunk = CHUNK_ROWS key/value rows laid out so each
# SBUF partition holds CHUNK_ROWS/128 *consecutive* DRAM rows (contiguous
# per-partition DMA descriptors). Keys and values use the same permutation, so
# the softmax/matmul pairing stays consistent.
CHUNK_ROWS = 512
SUBT = CHUNK_ROWS // 128  # sub-tiles ([128,512] slabs) per chunk


MM_MODE = "f32"  # "f32" | "f32r"
NACT = 4  # sub-tiles/chunk whose abs+reduce runs on ScalarE; rest on VectorE
KBUFS = 3
VBUFS = 3
DBUFS = 3
SKIP_COMPUTE_A = False  # DMA keys but skip score compute (perf probes)
SKIP_STATS = False
FAKE_MIN_CHAIN = False  # exp with immediate bias (perf probe)
SKIP_PHASE_B = False


FUSED = True


def _body_fused(ctx, tc, q, k, v, ident, ones, out_vec, out_stats, shard_rows, chunk_rows):
    """Single fused stream: keys and values chunks flow together.

    The softmax reference point m is the min over chunk 0's scores (any
    reference is mathematically valid; host combine only needs consistency).
    exp(m - t) <= exp(m - min t) stays far below fp32 max (~e^20 worst case),
    so per-chunk exp + matmul can run as soon as each chunk's scores land —
    no global barrier, both DMA streams stay saturated end to end.
    """
    nc = tc.nc
    subt = chunk_rows // 128
    nchunk = shard_rows // chunk_rows
    ncols = shard_rows // 128
    nact = min(NACT, subt)

    singles = ctx.enter_context(tc.tile_pool(name="singles", bufs=1))
    kpool = ctx.enter_context(tc.tile_pool(name="kpool", bufs=KBUFS))
    vpool = ctx.enter_context(tc.tile_pool(name="vpool", bufs=VBUFS))
    dpool = ctx.enter_context(tc.tile_pool(name="dpool", bufs=DBUFS))
    apool = ctx.enter_context(tc.tile_pool(name="apool", bufs=4))
    pp = ctx.enter_context(tc.tile_pool(name="pp", bufs=1, space="PSUM"))

    qb = singles.tile([128, F], F32, tag="qb")
    nc.sync.dma_start(
        out=qb,
        in_=bass.AP(tensor=q.tensor, offset=q.offset, ap=[[0, 128]] + list(q.ap)),
    )
    id_sb = singles.tile([128, 128], F32, tag="id")
    nc.sync.dma_start(out=id_sb, in_=ident)
    ones_sb = singles.tile([1, 128], F32, tag="ones")
    nc.sync.dma_start(out=ones_sb, in_=ones)

    scores = singles.tile([128, ncols], F32, tag="scores")
    att = singles.tile([128, ncols], F32, tag="att")
    gmin = singles.tile([1, 1], F32, tag="gmin")
    gmin_col = singles.tile([128, 1], F32, tag="gmincol")

    kt = k.rearrange("(n p j) f -> n p j f", p=128, j=subt)
    vt = v.rearrange("(n p j) f -> n p j f", p=128, j=subt)

    acc = pp.tile([1, F], F32, tag="acc")
    for n in range(nchunk):
        kc = kpool.tile([128, subt, F], F32, tag="kc")
        nc.sync.dma_start(out=kc, in_=kt[n])
        vc = vpool.tile([128, subt, F], F32, tag="vc")
        nc.scalar.dma_start(out=vc, in_=vt[n])

        dchunk = dpool.tile([128, subt, F], F32, tag="dch")
        for j in range(subt):
            nc.vector.tensor_tensor(
                out=dchunk[:, j], in0=kc[:, j], in1=qb, op=mybir.AluOpType.subtract
            )
        col0 = n * subt
        for j in range(nact):
            a = apool.tile([128, F], F32, tag="a")
            nc.scalar.activation(
                out=a,
                in_=dchunk[:, j],
                func=mybir.ActivationFunctionType.Abs,
                accum_out=scores[:, col0 + j : col0 + j + 1],
            )
        if nact < subt:
            nc.vector.tensor_reduce(
                out=scores[:, col0 + nact : col0 + subt],
                in_=dchunk[:, nact:subt],
                axis=mybir.AxisListType.X,
                op=mybir.AluOpType.add,
                apply_absolute_value=True,
            )

        if n == 0:
            # Softmax reference point m = (min over chunk 0) - MARGIN. Any
            # reference is valid for the host combine; the margin guards
            # exp overflow if a later chunk has t far below chunk 0's min
            # (underflow of negligible weights is harmless).
            tmin = singles.tile([128, 1], F32, tag="tmin")
            nc.vector.tensor_reduce(
                out=tmin,
                in_=scores[:, 0:subt],
                axis=mybir.AxisListType.X,
                op=mybir.AluOpType.min,
            )
            trow_ps = pp.tile([1, 128], F32, tag="trow")
            nc.tensor.transpose(trow_ps, tmin, id_sb)
            nc.vector.tensor_reduce(
                out=gmin,
                in_=trow_ps,
                axis=mybir.AxisListType.X,
                op=mybir.AluOpType.min,
            )
            nc.vector.tensor_scalar_add(gmin, gmin, -20.0)
            bc_ps = pp.tile([128, 1], F32, tag="bc")
            nc.tensor.matmul(bc_ps, lhsT=ones_sb, rhs=gmin, start=True, stop=True)
            nc.scalar.copy(out=gmin_col, in_=bc_ps)

        nc.scalar.activation(
            out=att[:, col0 : col0 + subt],
            in_=scores[:, col0 : col0 + subt],
            func=mybir.ActivationFunctionType.Exp,
            bias=gmin_col,
            scale=-1.0,
        )
        for j in range(subt):
            c = col0 + j
            lhs_ap = att[:, c : c + 1]
            rhs_ap = vc[:, j]
            if MM_MODE == "f32r":
                lhs_ap = lhs_ap.bitcast(mybir.dt.float32r)
                rhs_ap = rhs_ap.bitcast(mybir.dt.float32r)
            nc.tensor.matmul(
                acc,
                lhsT=lhs_ap,
                rhs=rhs_ap,
                start=(c == 0),
                stop=(c == ncols - 1),
            )

    # ---- tail: s = sum(att), pack outputs ----
    scol = singles.tile([128, 1], F32, tag="scol")
    nc.vector.tensor_reduce(
        out=scol, in_=att, axis=mybir.AxisListType.X, op=mybir.AluOpType.add
    )
    srow_ps = pp.tile([1, 128], F32, tag="srow")
    nc.tensor.transpose(srow_ps, scol, id_sb)
    ssum = singles.tile([1, 1], F32, tag="ssum")
    nc.vector.tensor_reduce(
        out=ssum, in_=srow_ps, axis=mybir.AxisListType.X, op=mybir.AluOpType.add
    )
    out_sb = singles.tile([1, F], F32, tag="outsb")
    nc.vector.tensor_copy(out=out_sb, in_=acc)
    st_sb = singles.tile([1, 2], F32, tag="stsb")
    nc.vector.tensor_copy(out=st_sb[:, 0:1], in_=ssum)
    nc.vector.tensor_copy(out=st_sb[:, 1:2], in_=gmin)
    nc.sync.dma_start(out=out_vec, in_=out_sb)
    nc.sync.dma_start(out=out_stats, in_=st_sb)


def _body(ctx, tc, q, k, v, ident, ones, out_vec, out_stats, shard_rows, chunk_rows):
    if FUSED:
        return _body_fused(
            ctx, tc, q, k, v, ident, ones, out_vec, out_stats, shard_rows, chunk_rows
        )
    nc = tc.nc
    subt = chunk_rows // 128
    nchunk = shard_rows // chunk_rows
    ncols = shard_rows // 128  # score columns

    singles = ctx.enter_context(tc.tile_pool(name="singles", bufs=1))
    kpool = ctx.enter_context(tc.tile_pool(name="kpool", bufs=3))
    vpool = ctx.enter_context(tc.tile_pool(name="vpool", bufs=3))
    dpool = ctx.enter_context(tc.tile_pool(name="dpool", bufs=3))
    apool = ctx.enter_context(tc.tile_pool(name="apool", bufs=4))
    pp = ctx.enter_context(tc.tile_pool(name="pp", bufs=1, space="PSUM"))

    # query broadcast to all 128 partitions (stride-0 partition DMA)
    qb = singles.tile([128, F], F32, tag="qb")
    nc.sync.dma_start(
        out=qb,
        in_=bass.AP(tensor=q.tensor, offset=q.offset, ap=[[0, 128]] + list(q.ap)),
    )
    id_sb = singles.tile([128, 128], F32, tag="id")
    nc.sync.dma_start(out=id_sb, in_=ident)
    ones_sb = singles.tile([1, 128], F32, tag="ones")
    nc.sync.dma_start(out=ones_sb, in_=ones)

    scores = singles.tile([128, ncols], F32, tag="scores")
    if SKIP_COMPUTE_A:
        nc.vector.memset(scores, 1.0)

    kt = k.rearrange("(n p j) f -> n p j f", p=128, j=subt)
    vt = v.rearrange("(n p j) f -> n p j f", p=128, j=subt)

    # ---- Phase A: stream keys, compute t_c = sum_f |k - q| ----
    for n in range(nchunk):
        kc = kpool.tile([128, subt, F], F32, tag="kc")
        nc.sync.dma_start(out=kc, in_=kt[n])
        if SKIP_COMPUTE_A:
            continue
        nact = min(NACT, subt)
        dchunk = dpool.tile([128, subt, F], F32, tag="dch")
        for j in range(subt):
            nc.vector.tensor_tensor(
                out=dchunk[:, j], in0=kc[:, j], in1=qb, op=mybir.AluOpType.subtract
            )
        for j in range(nact):
            c = n * subt + j
            a = apool.tile([128, F], F32, tag="a")
            nc.scalar.activation(
                out=a,
                in_=dchunk[:, j],
                func=mybir.ActivationFunctionType.Abs,
                accum_out=scores[:, c : c + 1],
            )
        if nact < subt:
            nc.vector.tensor_reduce(
                out=scores[:, n * subt + nact : n * subt + subt],
                in_=dchunk[:, nact:subt],
                axis=mybir.AxisListType.X,
                op=mybir.AluOpType.add,
                apply_absolute_value=True,
            )

    # ---- Stats: global min of t, broadcast, exponentiate ----
    att = singles.tile([128, ncols], F32, tag="att")
    gmin = singles.tile([1, 1], F32, tag="gmin")
    ssum = singles.tile([1, 1], F32, tag="ssum")
    if SKIP_STATS:
        nc.vector.memset(att, 0.5)
        nc.vector.memset(gmin, 0.0)
        nc.vector.memset(ssum, 1.0)
    else:
        tmin = singles.tile([128, 1], F32, tag="tmin")
        nc.vector.tensor_reduce(
            out=tmin, in_=scores, axis=mybir.AxisListType.X, op=mybir.AluOpType.min
        )
        trow_ps = pp.tile([1, 128], F32, tag="trow")
        nc.tensor.transpose(trow_ps, tmin, id_sb)
        nc.vector.tensor_reduce(
            out=gmin, in_=trow_ps, axis=mybir.AxisListType.X, op=mybir.AluOpType.min
        )
        bc_ps = pp.tile([128, 1], F32, tag="bc")
        nc.tensor.matmul(bc_ps, lhsT=ones_sb, rhs=gmin, start=True, stop=True)
        gmin_col = singles.tile([128, 1], F32, tag="gmincol")
        nc.scalar.copy(out=gmin_col, in_=bc_ps)

        nc.scalar.activation(
            out=att,
            in_=scores,
            func=mybir.ActivationFunctionType.Exp,
            bias=0.0 if FAKE_MIN_CHAIN else gmin_col,
            scale=-1.0,
        )
        scol = singles.tile([128, 1], F32, tag="scol")
        nc.vector.tensor_reduce(
            out=scol, in_=att, axis=mybir.AxisListType.X, op=mybir.AluOpType.add
        )
        srow_ps = pp.tile([1, 128], F32, tag="srow")
        nc.tensor.transpose(srow_ps, scol, id_sb)
        nc.vector.tensor_reduce(
            out=ssum, in_=srow_ps, axis=mybir.AxisListType.X, op=mybir.AluOpType.add
        )

    # ---- Phase B: stream values, accumulate att @ V into one PSUM bank ----
    acc = None if SKIP_PHASE_B else pp.tile([1, F], F32, tag="acc")
    for n in range(nchunk):
        vc = vpool.tile([128, subt, F], F32, tag="vc")
        nc.scalar.dma_start(out=vc, in_=vt[n])
        if SKIP_PHASE_B:
            continue
        for j in range(subt):
            c = n * subt + j
            lhs_ap = att[:, c : c + 1]
            rhs_ap = vc[:, j]
            if MM_MODE == "f32r":
                lhs_ap = lhs_ap.bitcast(mybir.dt.float32r)
                rhs_ap = rhs_ap.bitcast(mybir.dt.float32r)
            nc.tensor.matmul(
                acc,
                lhsT=lhs_ap,
                rhs=rhs_ap,
                start=(c == 0),
                stop=(c == ncols - 1),
            )

    out_sb = singles.tile([1, F], F32, tag="outsb")
    if SKIP_PHASE_B:
        nc.vector.memset(out_sb, 0.0)
    else:
        nc.vector.tensor_copy(out=out_sb, in_=acc)
    st_sb = singles.tile([1, 2], F32, tag="stsb")
    nc.vector.tensor_copy(out=st_sb[:, 0:1], in_=ssum)
    nc.vector.tensor_copy(out=st_sb[:, 1:2], in_=gmin)
    nc.sync.dma_start(out=out_vec, in_=out_sb)
    nc.sync.dma_start(out=out_stats, in_=st_sb)


def build_nc(shard_rows=SHARD, chunk_rows=CHUNK_ROWS, num_devices=NCORES, reps=1):
    nc = bacc.Bacc(
        "TRN2", target_bir_lowering=False, debug=False, num_devices=num_devices
    )
    q_h = nc.dram_tensor("query", [F], F32, kind="ExternalInput")
    k_h = nc.dram_tensor("keys", [shard_rows, F], F32, kind="ExternalInput")
    v_h = nc.dram_tensor("values", [shard_rows, F], F32, kind="ExternalInput")
    id_h = nc.dram_tensor("ident", [128, 128], F32, kind="ExternalInput")
    ones_h = nc.dram_tensor("ones_row", [1, 128], F32, kind="ExternalInput")
    onum_h = nc.dram_tensor("out_vec", [1, F], F32, kind="ExternalOutput")
    ostat_h = nc.dram_tensor("out_stats", [1, 2], F32, kind="ExternalOutput")

    with tile.TileContext(nc) as tc, ExitStack() as ctx:
        for _ in range(reps):
            with ExitStack() as rep_ctx:
                _body(
                    rep_ctx,
                    tc,
                    q_h.ap(),
                    k_h.ap(),
                    v_h.ap(),
                    id_h.ap(),
                    ones_h.ap(),
                    onum_h.ap(),
                    ostat_h.ap(),
                    shard_rows,
                    chunk_rows,
                )
    nc.compile()
    return nc


def make_in_maps(query, keys, values, shard_rows=SHARD, ncores=NCORES):
    query = np.ascontiguousarray(np.asarray(query), dtype=np.float32)
    keys = np.asarray(keys)
    values = np.asarray(values)
    ident = np.eye(128, dtype=np.float32)
    ones = np.ones((1, 128), dtype=np.float32)
    in_maps = []
    for i in range(ncores):
        sl = slice(i * shard_rows, (i + 1) * shard_rows)
        in_maps.append(
            {
                "query": query,
                "keys": np.ascontiguousarray(keys[sl], dtype=np.float32),
                "values": np.ascontiguousarray(values[sl], dtype=np.float32),
                "ident": ident,
                "ones_row": ones,
            }
        )
    return in_maps


def combine(results):
    """Merge per-core (num, s, tmin) partials: stable cross-shard softmax."""
    num = np.stack([np.asarray(r["out_vec"])[0] for r in results]).astype(np.float64)
    st = np.stack([np.asarray(r["out_stats"])[0] for r in results]).astype(np.float64)
    s, tmin = st[:, 0], st[:, 1]
    t0 = tmin.min()
    w = np.exp(t0 - tmin)  # <= 1
    vec = (num * w[:, None]).sum(axis=0) / (s * w).sum()
    return (1.0 / (1.0 + np.exp(-vec))).astype(np.float32)


_NC_CACHE = None


def kernel(query, keys, values):
    global _NC_CACHE
    if _NC_CACHE is None:
        _NC_CACHE = build_nc()
    in_maps = make_in_maps(query, keys, values)
    res = run_bass_kernel_spmd(_NC_CACHE, in_maps, core_ids=list(range(NCORES)))
    return combine(res.results)


if __name__ == "__main__":
    rng = np.random.default_rng(0)
    q = rng.standard_normal(F).astype(np.float32)
    k = rng.standard_normal((CAP, F)).astype(np.float32)
    v = rng.standard_normal((CAP, F)).astype(np.float32)
    out = kernel(q, k, v)
    print(out[:8])
